# revision 13
# baseline (speedup 1.0000x reference)
"""Call-guided sparse attention kernel for Trainium2 (8 NeuronCores).

Sharding: batch (4) x head-group (2 groups of 4 heads) -> 8 cores.
Each core computes, for its batch element b and its 4 heads:
  - q4[h]: Q projection zero-padded per head (via zero-padded weights) so
    per-head scores are full K=128 contractions against KfT
  - KfT (full D, own-heads-first permuted), v4: per-head zero-padded V
  - routing scores Sc = Qc_full . Kf_full for caller rows (opcode==0),
    top-16 threshold per caller row via vector.max + match_replace
  - banded window attention (|i-j|<=50) for ALL rows
  - dense union-masked (window | top16) attention for caller rows
  - output projection with its half of Wo (host sums the two partials)
"""

import os
import sys

import numpy as np

for _p in ("/opt/trn_rl_repo", "/root/.axon_site/_ro/trn_rl_repo"):
    if os.path.isdir(_p) and _p not in sys.path:
        sys.path.insert(0, _p)

import concourse.bass as bass
import concourse.mybir as mybir
from concourse import bacc
from concourse.tile import TileContext
from concourse.bass_utils import run_bass_kernel_spmd

F32 = mybir.dt.float32
F16 = mybir.dt.float16
AF = mybir.ActivationFunctionType
ALU = mybir.AluOpType

B, S, D, H = 4, 2048, 256, 8
DK = D // H          # 32
HPC = H // 2         # 4 heads per core
DH = HPC * DK        # 128 context dims per core
WINDOW = 50
TOPK = 16
NCAP = 384           # padded caller-row capacity per batch element
DA = D + 1           # bias-augmented contraction dim
SCALE = 1.0 / np.sqrt(np.float32(DK))
NT = S // 128        # 16 row tiles
NM = NCAP // 128     # 3 caller-row tiles


def _build_program(stage=4):
    nc = bacc.Bacc("TRN2", target_bir_lowering=False, debug=False,
                   num_devices=8)

    # ---- DRAM I/O ----
    xT = nc.dram_tensor("xT", [DA, S], F32, kind="ExternalInput")
    xcT = nc.dram_tensor("xcT", [DA, NCAP], F32, kind="ExternalInput")
    xTh = nc.dram_tensor("xTh", [DA, S], F16, kind="ExternalInput")
    wq4 = nc.dram_tensor("wq4", [DA, HPC * 128], F16, kind="ExternalInput")
    wqf = nc.dram_tensor("wqf", [DA, D], F32, kind="ExternalInput")
    wkf = nc.dram_tensor("wkf", [DA, D], F32, kind="ExternalInput")
    wv4 = nc.dram_tensor("wv4", [DA, HPC * 128], F16, kind="ExternalInput")
    woh = nc.dram_tensor("woh", [DH, D], F16, kind="ExternalInput")
    ci_col = nc.dram_tensor("ci_col", [NCAP, 1], F32, kind="ExternalInput")
    pada = nc.dram_tensor("pada", [128, NT], F32, kind="ExternalInput")
    w01d = nc.dram_tensor("w01", [3, 128, 128], F16, kind="ExternalInput")
    e4d = nc.dram_tensor("e4", [HPC, 128], F32, kind="ExternalInput")
    identd = nc.dram_tensor("ident", [128, 128], F32, kind="ExternalInput")
    outT = nc.dram_tensor("outT", [D, S], F32, kind="ExternalOutput")
    outcT = nc.dram_tensor("outcT", [D, NCAP], F32, kind="ExternalOutput")

    with TileContext(nc) as tc:
        with (
            tc.tile_pool(name="const", bufs=1) as cst,
            tc.tile_pool(name="persist", bufs=1) as per,
            tc.tile_pool(name="mbig", bufs=1) as mbig,
            tc.tile_pool(name="alp", bufs=1) as alp,
            tc.tile_pool(name="wrk", bufs=3) as wrk,
        ):
            # ---------- small constants ----------
            wq4_sb, wqf_sb, wkf_sb, wv4_sb = [], [], [], []
            for k, (lo, hi) in enumerate(((0, 128), (128, 256), (256, 257))):
                p = hi - lo
                t4 = cst.tile([p, HPC * 128], F16, tag=f"wq4_{k}",
                              name=f"wq4_{k}")
                tq = cst.tile([p, D], F32, tag=f"wqf_{k}", name=f"wqf_{k}")
                tk = cst.tile([p, D], F32, tag=f"wkf_{k}", name=f"wkf_{k}")
                tv = cst.tile([p, HPC * 128], F16, tag=f"wv4_{k}",
                              name=f"wv4_{k}")
                nc.sync.dma_start(t4[:], wq4[lo:hi, :])
                nc.sync.dma_start(tq[:], wqf[lo:hi, :])
                nc.sync.dma_start(tk[:], wkf[lo:hi, :])
                nc.sync.dma_start(tv[:], wv4[lo:hi, :])
                wq4_sb.append(t4)
                wqf_sb.append(tq)
                wkf_sb.append(tk)
                wv4_sb.append(tv)
            woh_sb = cst.tile([DH, D], F16, tag="woh")
            nc.sync.dma_start(woh_sb[:], woh[:])

            ci_sb = []
            for m in range(NM):
                t = cst.tile([128, 1], F32, tag=f"ci{m}", name=f"ci{m}")
                nc.sync.dma_start(t[:], ci_col[m * 128:(m + 1) * 128, :])
                ci_sb.append(t)
            pada_sb = cst.tile([128, NT], F32, tag="pada")
            nc.sync.dma_start(pada_sb[:], pada[:])

            w01_sb = []
            for k in range(3):
                t = cst.tile([128, 1, 128], F16, tag=f"w01{k}", name=f"w01{k}")
                nc.sync.dma_start(t[:, 0, :], w01d[k])
                w01_sb.append(t)
            e4_sb = cst.tile([HPC, 128], F32, tag="e4")
            nc.sync.dma_start(e4_sb[:], e4d[:])
            ident_sb = cst.tile([128, 128], F32, tag="ident")
            nc.sync.dma_start(ident_sb[:], identd[:])

            ones128 = cst.tile([128, 1], F32, tag="ones128")
            nc.vector.memset(ones128[:], 1.0)
            ones128h = cst.tile([128, 1], F16, tag="ones128h")
            nc.vector.memset(ones128h[:], 1.0)
            ones1 = cst.tile([1, 128], F32, tag="ones1")
            nc.vector.memset(ones1[:], 1.0)

            # persistent activations
            q4 = [per.tile([128, S], F16, tag=f"q4_{h}", name=f"q4_{h}")
                  for h in range(HPC)]
            kft = [per.tile([128, S], F32, tag=f"kft{m}", name=f"kft{m}")
                   for m in range(2)]
            kfth = per.tile([128, S], F16, tag="kfth")
            qct = [per.tile([128, NCAP], F32, tag=f"qct{m}", name=f"qct{m}")
                   for m in range(2)]
            qc4 = per.tile([128, HPC, NCAP], F16, tag="qc4")
            v4 = [per.tile([128, HPC * 128], F16, tag=f"v4_{j}",
                           name=f"v4_{j}") for j in range(NT)]
            alT_sb = [alp.tile([128, 1, NCAP], F16, tag=f"alT{j}",
                               name=f"alT{j}") for j in range(NT)]

            with (
                tc.tile_pool(name="load", bufs=1) as ld,
                tc.tile_pool(name="psmm", bufs=2, space="PSUM") as psmm,
                tc.tile_pool(name="bps", bufs=2, space="PSUM") as bps,
                tc.tile_pool(name="bacc", bufs=2, space="PSUM") as bap,
                tc.tile_pool(name="bwork", bufs=2) as bwrk,
            ):
                # ---------- load x ----------
                xt0 = ld.tile([128, S], F32, tag="xt0")
                xt1 = ld.tile([128, S], F32, tag="xt1")
                xt2 = ld.tile([1, S], F32, tag="xt2")
                nc.sync.dma_start(xt0[:], xT[0:128, :])
                nc.sync.dma_start(xt1[:], xT[128:256, :])
                nc.sync.dma_start(xt2[:], xT[256:257, :])
                xct0 = ld.tile([128, NCAP], F32, tag="xct0")
                xct1 = ld.tile([128, NCAP], F32, tag="xct1")
                xct2 = ld.tile([1, NCAP], F32, tag="xct2")
                nc.sync.dma_start(xct0[:], xcT[0:128, :])
                nc.sync.dma_start(xct1[:], xcT[128:256, :])
                nc.sync.dma_start(xct2[:], xcT[256:257, :])
                xts = (xt0, xt1, xt2)
                xcts = (xct0, xct1, xct2)
                xh0 = ld.tile([128, S], F16, tag="xh0")
                xh1 = ld.tile([128, S], F16, tag="xh1")
                xh2 = ld.tile([1, S], F16, tag="xh2")
                nc.sync.dma_start(xh0[:], xTh[0:128, :])
                nc.sync.dma_start(xh1[:], xTh[128:256, :])
                nc.sync.dma_start(xh2[:], xTh[256:257, :])
                xhs = (xh0, xh1, xh2)
                xch = ld.tile([128, HPC, NCAP], F16, tag="xch")
                al_t = ld.tile([128, S], F32, tag="al", name="al_t")

                # ---------- projections ----------
                # q4[h]: per-head zero-padded Q (own heads)
                for h in range(HPC):
                    hsl = bass.ts(h, 128)
                    for c in range(4):
                        ps = psmm.tile([128, 512], F32, tag="mm")
                        sl = bass.ts(c, 512)
                        for k in range(3):
                            nc.tensor.matmul(ps[:], wq4_sb[k][:, hsl],
                                             xhs[k][:, sl],
                                             start=(k == 0), stop=(k == 2))
                        nc.scalar.activation(q4[h][:, sl], ps[:], AF.Copy)

                # KfT full [2][128, S]
                for m in range(2):
                    msl = bass.ts(m, 128)
                    for c in range(4):
                        ps = psmm.tile([128, 512], F32, tag="mm")
                        sl = bass.ts(c, 512)
                        for k in range(3):
                            nc.tensor.matmul(ps[:], wkf_sb[k][:, msl],
                                             xts[k][:, sl],
                                             start=(k == 0), stop=(k == 2))
                        nc.scalar.activation(kft[m][:, sl], ps[:], AF.Copy)
                        if m == 0:
                            nc.scalar.activation(kfth[:, sl], ps[:], AF.Copy)

                # v4: per-head zero-padded V, natural layout
                for jt in range(NT):
                    sl = bass.ts(jt, 128)
                    ps = psmm.tile([128, 512], F32, tag="mm")
                    for k in range(3):
                        nc.tensor.matmul(ps[:], xhs[k][:, sl], wv4_sb[k][:],
                                         start=(k == 0), stop=(k == 2))
                    nc.scalar.activation(v4[jt][:], ps[:], AF.Copy)

                # QcT full [2][128, NCAP] (routing) + qc4 (per-head padded)
                for m in range(2):
                    msl = bass.ts(m, 128)
                    ps = psmm.tile([128, NCAP], F32, tag="mm")
                    for k in range(3):
                        nc.tensor.matmul(ps[:], wqf_sb[k][:, msl], xcts[k][:],
                                         start=(k == 0), stop=(k == 2))
                    nc.scalar.activation(qct[m][:], ps[:], AF.Copy)
                nc.scalar.activation(xch[:, 0, :], xct0[:], AF.Copy)
                nc.scalar.activation(xch[:, 1, :], xct1[:], AF.Copy)
                for h in range(HPC):
                    hsl = bass.ts(h, 128)
                    ps = psmm.tile([128, NCAP], F32, tag="mm")
                    nc.tensor.matmul(ps[:], wq4_sb[0][:, hsl], xch[:, 0, :],
                                     start=True, stop=False)
                    nc.tensor.matmul(ps[:], wq4_sb[1][:, hsl], xch[:, 1, :],
                                     start=False, stop=True)
                    nc.scalar.activation(qc4[:, h, :], ps[:], AF.Copy)

                if stage >= 2:
                    # ------- routing scores + topk + union mask -------
                    for mt in range(NM):
                        sc = mbig.tile([128, S], F32, tag="sc")
                        msl = bass.ts(mt, 128)
                        for c in range(4):
                            ps = psmm.tile([128, 512], F32, tag="mm")
                            sl = bass.ts(c, 512)
                            nc.tensor.matmul(ps[:], qct[0][:, msl],
                                             kft[0][:, sl],
                                             start=True, stop=False)
                            nc.tensor.matmul(ps[:], qct[1][:, msl],
                                             kft[1][:, sl],
                                             start=False, stop=True)
                            nc.scalar.activation(sc[:, sl], ps[:], AF.Copy)

                        m8a = wrk.tile([128, 8], F32, tag="m8a")
                        m8b = wrk.tile([128, 8], F32, tag="m8b")
                        tmp1 = mbig.tile([128, S], F32, tag="tmp1")
                        nc.vector.max(out=m8a[:], in_=sc[:])
                        nc.vector.match_replace(out=tmp1[:],
                                                in_to_replace=m8a[:],
                                                in_values=sc[:],
                                                imm_value=-1e30)
                        nc.vector.max(out=m8b[:], in_=tmp1[:])
                        # window part: al = ((j - i)^2 <= W^2)
                        nc.gpsimd.iota(tmp1[:], pattern=[[1, S]], base=0,
                                       channel_multiplier=0,
                                       allow_small_or_imprecise_dtypes=True)
                        nc.vector.tensor_scalar(tmp1[:], tmp1[:],
                                                ci_sb[mt][:], None,
                                                op0=ALU.subtract)
                        nc.scalar.activation(tmp1[:], tmp1[:], AF.Square)
                        nc.vector.tensor_scalar(al_t[:], tmp1[:],
                                                float(WINDOW * WINDOW), None,
                                                op0=ALU.is_le)
                        # union with guided: al = max(al, sc >= t16)
                        nc.vector.scalar_tensor_tensor(
                            out=al_t[:], in0=sc[:],
                            scalar=m8b[:, 7:8],
                            in1=al_t[:], op0=ALU.is_ge, op1=ALU.max)

                        # transpose allowed-mask into [j, i] tiles
                        for jt in range(NT):
                            jsl = bass.ts(jt, 128)
                            psal = psmm.tile([128, 128], F32, tag="mm")
                            nc.tensor.transpose(psal[:], al_t[:, jsl],
                                                ident_sb[:])
                            nc.scalar.activation(
                                alT_sb[jt][:, 0, bass.ts(mt, 128)],
                                psal[:], AF.Copy)

                if stage >= 3:
                    # ------- banded window attention (all rows) -------
                    for it in range(NT):
                        r0 = it * 128
                        if it == 0:
                            subs = [(0, 1), (1, 2)]
                        elif it == NT - 1:
                            subs = [(it - 1, 0), (it, 1)]
                        else:
                            subs = [(it - 1, 0), (it, 1), (it + 1, 2)]

                        bctx = bap.tile([128, 128], F32, tag="bctx")
                        bsums = bap.tile([1, HPC, 128], F32, tag="bsums")
                        nsub = len(subs)
                        for si, (jt, wk_id) in enumerate(subs):
                            jsl = bass.ts(jt, 128)
                            ps = bps.tile([128, HPC, 128], F32, tag="bsc")
                            for h in range(HPC):
                                nc.tensor.matmul(
                                    ps[:, h, :], kfth[:, jsl],
                                    q4[h][:, bass.ts(it, 128)],
                                    start=True, stop=True)
                            e = bwrk.tile([128, HPC, 128], F16, tag="be")
                            nc.scalar.activation(e[:], ps[:], AF.Exp)
                            em = bwrk.tile([128, HPC, 128], F16, tag="bem")
                            nc.vector.scalar_tensor_tensor(
                                out=em[:], in0=e[:],
                                scalar=pada_sb[:, jt:jt + 1],
                                in1=w01_sb[wk_id][:].to_broadcast(
                                    (128, HPC, 128)),
                                op0=ALU.mult, op1=ALU.mult)
                            st = (si == 0)
                            sp = (si == nsub - 1)
                            nc.tensor.matmul(
                                bsums[:].rearrange("a h n -> a (h n)"),
                                ones128h[:],
                                em[:].rearrange("p h n -> p (h n)"),
                                start=st, stop=sp, skip_group_check=True)
                            for h in range(HPC):
                                nc.tensor.matmul(
                                    bctx[:], v4[jt][:, bass.ts(h, 128)],
                                    em[:, h, :],
                                    start=(st and h == 0),
                                    stop=(sp and h == HPC - 1),
                                    skip_group_check=True)

                        r1 = bwrk.tile([1, HPC, 128], F32, tag="br1")
                        nc.vector.reciprocal(r1[:], bsums[:])
                        r4 = bwrk.tile([HPC, 128], F32, tag="br4")
                        nc.sync.dma_start(r4[:], r1[0:1, :, :])
                        psrb = psmm.tile([128, 128], F32, tag="mm")
                        nc.tensor.matmul(psrb[:], e4_sb[:], r4[:],
                                         start=True, stop=True)
                        rb_sb = bwrk.tile([128, 128], F32, tag="brb")
                        nc.scalar.activation(rb_sb[:], psrb[:], AF.Copy)
                        ctx_sb = bwrk.tile([128, 128], F16, tag="bctxs")
                        nc.vector.tensor_mul(ctx_sb[:], bctx[:], rb_sb[:])
                        pso = psmm.tile([128, 2, 128], F32, tag="mm")
                        for m in range(2):
                            nc.tensor.matmul(pso[:, m, :],
                                             woh_sb[:, bass.ts(m, 128)],
                                             ctx_sb[:], start=True, stop=True)
                        osb = bwrk.tile([128, 2, 128], F32, tag="osb")
                        nc.scalar.activation(osb[:], pso[:], AF.Copy)
                        for m in range(2):
                            nc.sync.dma_start(
                                outT[m * 128:(m + 1) * 128, r0:r0 + 128],
                                osb[:, m, :])

            if stage >= 4:
                # ---------- caller dense attention ----------
                with (
                    tc.tile_pool(name="cps", bufs=1, space="PSUM") as cps,
                    tc.tile_pool(name="cacc", bufs=1, space="PSUM") as cacc,
                    tc.tile_pool(name="cwork", bufs=2) as cwrk,
                ):
                    cctx = cacc.tile([128, NCAP], F32, tag="cctx")
                    csums = cacc.tile([1, 3, 512], F32, tag="csums")
                    for jt in range(NT):
                        jsl = bass.ts(jt, 128)
                        st = (jt == 0)
                        sp = (jt == NT - 1)
                        ps = cps.tile([128, HPC, 512], F32, tag="csc")
                        for h in range(HPC):
                            nc.tensor.matmul(
                                ps[:, h, 0:NCAP], kfth[:, jsl],
                                qc4[:, h, :], start=True, stop=True)
                        e = cwrk.tile([128, HPC, NCAP], F16, tag="ce")
                        for h in range(HPC):
                            nc.scalar.activation(e[:, h, :], ps[:, h, 0:NCAP],
                                                 AF.Exp)
                        em = cwrk.tile([128, HPC, NCAP], F16, tag="cem")
                        nc.vector.scalar_tensor_tensor(
                            out=em[:], in0=e[:],
                            scalar=pada_sb[:, jt:jt + 1],
                            in1=alT_sb[jt][:].to_broadcast((128, HPC, NCAP)),
                            op0=ALU.mult, op1=ALU.mult)
                        emf = em[:].rearrange("p h n -> p (h n)")
                        for c in range(3):
                            nc.tensor.matmul(
                                csums[0:1, c, :], ones128h[:],
                                emf[:, bass.ts(c, 512)],
                                start=st, stop=sp, skip_group_check=True)
                        for h in range(HPC):
                            nc.tensor.matmul(
                                cctx[:], v4[jt][:, bass.ts(h, 128)],
                                em[:, h, :],
                                start=(st and h == 0),
                                stop=(sp and h == HPC - 1),
                                skip_group_check=True)

                    cr1 = cwrk.tile([1, 3, 512], F32, tag="cr1")
                    nc.vector.reciprocal(cr1[:], csums[:])
                    r4c = cwrk.tile([HPC, NCAP], F32, tag="cr4")
                    nc.sync.dma_start(
                        r4c[:],
                        cr1[0:1, :, :].rearrange(
                            "a c n -> a (c n)")[:, 0:HPC * NCAP])
                    pscrb = cps.tile([128, HPC, 512], F32, tag="csc")
                    nc.tensor.matmul(pscrb[:, 0, 0:NCAP], e4_sb[:], r4c[:],
                                     start=True, stop=True)
                    crb_sb = cwrk.tile([128, NCAP], F32, tag="crb")
                    nc.scalar.activation(crb_sb[:], pscrb[:, 0, 0:NCAP],
                                         AF.Copy)
                    cctx_sb = cwrk.tile([128, NCAP], F16, tag="cctxs")
                    nc.vector.tensor_mul(cctx_sb[:], cctx[:], crb_sb[:])
                    psoc = cps.tile([128, HPC, 512], F32, tag="csc")
                    for m in range(2):
                        nc.tensor.matmul(psoc[:, m, 0:NCAP],
                                         woh_sb[:, bass.ts(m, 128)],
                                         cctx_sb[:], start=True, stop=True)
                    ocsb = cwrk.tile([128, 2, NCAP], F32, tag="ocsb")
                    for m in range(2):
                        nc.scalar.activation(ocsb[:, m, :],
                                             psoc[:, m, 0:NCAP], AF.Copy)
                        nc.sync.dma_start(outcT[m * 128:(m + 1) * 128, :],
                                          ocsb[:, m, :])

    nc.compile()
    nc.finalize()
    return nc


_NC_CACHE = None


def _get_program():
    global _NC_CACHE
    if _NC_CACHE is None:
        _NC_CACHE = _build_program()
    return _NC_CACHE


def _host_prepare(x, Wq, bq, Wk, bk, Wv, bv, Wo, bo, opcode_types, pad_mask):
    """Build per-core input dicts + metadata for unsharding."""
    x = np.ascontiguousarray(np.asarray(x, np.float32))
    Wq = np.asarray(Wq, np.float32)
    bq = np.asarray(bq, np.float32)
    Wk = np.asarray(Wk, np.float32)
    bk = np.asarray(bk, np.float32)
    Wv = np.asarray(Wv, np.float32)
    bv = np.asarray(bv, np.float32)
    Wo = np.asarray(Wo, np.float32)
    opcode = np.asarray(opcode_types)
    pad = np.asarray(pad_mask)

    wq_aug = np.vstack([Wq * SCALE, (bq * SCALE)[None, :]])     # [257, 256]
    wk_aug = np.vstack([Wk, bk[None, :]])
    wv_aug = np.vstack([Wv, bv[None, :]])

    w01 = np.zeros((3, 128, 128), np.float16)
    for k, base in enumerate((-128, 0, 128)):
        pj = np.arange(128)[:, None]
        pi = np.arange(128)[None, :]
        w01[k] = (np.abs(base + pj - pi) <= WINDOW).astype(np.float16)
    e4 = np.zeros((HPC, 128), np.float32)
    for h in range(HPC):
        e4[h, h * DK:(h + 1) * DK] = 1.0
    ident = np.eye(128, dtype=np.float32)

    in_maps = []
    meta = []
    for b in range(B):
        cidx = np.where(opcode[b] == 0)[0]
        nrows = len(cidx)
        if nrows > NCAP:
            raise RuntimeError(f"caller rows {nrows} exceed capacity {NCAP}")
        xc = np.zeros((NCAP, D), np.float32)
        xc[:nrows] = x[b, cidx]
        xc_aug = np.concatenate([xc, np.zeros((NCAP, 1), np.float32)], axis=1)
        xc_aug[:nrows, D] = 1.0
        ci = np.full((NCAP, 1), -1e6, np.float32)
        ci[:nrows, 0] = cidx.astype(np.float32)
        xT_aug = np.concatenate([x[b].T, np.ones((1, S), np.float32)], axis=0)
        pad01 = (pad[b] != 0).astype(np.float32)
        pada_arr = pad01.reshape(NT, 128).T.copy()

        meta.append((cidx, nrows))
        for hg in range(2):
            own = np.arange(hg * DH, (hg + 1) * DH)
            rest = np.setdiff1d(np.arange(D), own)
            perm = np.concatenate([own, rest])
            # per-head zero-padded Q / V weight blocks
            wq4_arr = np.zeros((DA, HPC * 128), np.float32)
            wv4_arr = np.zeros((DA, HPC * 128), np.float32)
            for h in range(HPC):
                csl = slice(hg * DH + h * DK, hg * DH + (h + 1) * DK)
                wq4_arr[:, h * 128 + h * DK:h * 128 + (h + 1) * DK] = \
                    wq_aug[:, csl]
                wv4_arr[:, h * 128 + h * DK:h * 128 + (h + 1) * DK] = \
                    wv_aug[:, csl]
            in_maps.append({
                "xT": np.ascontiguousarray(xT_aug),
                "xTh": np.ascontiguousarray(xT_aug.astype(np.float16)),
                "xcT": np.ascontiguousarray(xc_aug.T),
                "wq4": wq4_arr.astype(np.float16),
                "wqf": np.ascontiguousarray(wq_aug[:, perm]),
                "wkf": np.ascontiguousarray(wk_aug[:, perm]),
                "wv4": wv4_arr.astype(np.float16),
                "woh": np.ascontiguousarray(Wo[own, :].astype(np.float16)),
                "ci_col": ci,
                "pada": np.ascontiguousarray(pada_arr),
                "w01": w01,
                "e4": e4,
                "ident": ident,
            })
    return in_maps, meta


def _assemble(results, meta, bo):
    bo = np.asarray(bo, np.float32)
    out = np.empty((B, S, D), np.float32)
    for b in range(B):
        cidx, nrows = meta[b]
        full = results[2 * b]["outT"].T + results[2 * b + 1]["outT"].T
        if nrows > 0:
            oc = (results[2 * b]["outcT"].T +
                  results[2 * b + 1]["outcT"].T)[:nrows]
            full[cidx] = oc
        out[b] = full + bo[None, :]
    return out


def kernel(x, Wq, bq, Wk, bk, Wv, bv, Wo, bo, opcode_types, pad_mask,
           _trace=False):
    nc = _get_program()
    in_maps, meta = _host_prepare(x, Wq, bq, Wk, bk, Wv, bv, Wo, bo,
                                  opcode_types, pad_mask)
    res = run_bass_kernel_spmd(nc, in_maps, core_ids=list(range(8)),
                               trace=_trace)
    out = _assemble(res.results, meta, bo)
    if _trace:
        kernel.last_exec_time_ns = res.exec_time_ns
        kernel.last_results = res
    return out


# revision 22
# speedup vs baseline: 346.5332x; 346.5332x over previous
"""Call-guided sparse attention kernel for Trainium2 (8 NeuronCores).

Sharding: batch (4) x head-group (2 groups of 4 heads) -> 8 cores.
Each core computes, for its batch element b and its 4 heads:
  - q4[h]: Q projection zero-padded per head (via zero-padded weights) so
    per-head scores are full K=128 contractions against KfT
  - KfT (full D, own-heads-first permuted), v4: per-head zero-padded V
  - routing scores Sc = Qc_full . Kf_full for caller rows (opcode==0),
    top-16 threshold per caller row via vector.max + match_replace
  - banded window attention (|i-j|<=50) for ALL rows
  - dense union-masked (window | top16) attention for caller rows
  - output projection with its half of Wo (host sums the two partials)
"""

import os
import sys

import numpy as np

for _p in ("/opt/trn_rl_repo", "/root/.axon_site/_ro/trn_rl_repo"):
    if os.path.isdir(_p) and _p not in sys.path:
        sys.path.insert(0, _p)

import concourse.bass as bass
import concourse.mybir as mybir
from concourse import bacc
from concourse.tile import TileContext
from concourse.bass_utils import run_bass_kernel_spmd

F32 = mybir.dt.float32
F16 = mybir.dt.float16
AF = mybir.ActivationFunctionType
ALU = mybir.AluOpType

B, S, D, H = 4, 2048, 256, 8
DK = D // H          # 32
HPC = H // 2         # 4 heads per core
DH = HPC * DK        # 128 context dims per core
WINDOW = 50
TOPK = 16
NCAP = 384           # padded caller-row capacity per batch element
DA = D + 1           # bias-augmented contraction dim
SCALE = 1.0 / np.sqrt(np.float32(DK))
NT = S // 128        # 16 row tiles
NM = NCAP // 128     # 3 caller-row tiles


def _build_program(stage=4):
    nc = bacc.Bacc("TRN2", target_bir_lowering=False, debug=False,
                   num_devices=8)

    # ---- DRAM I/O ----
    xT = nc.dram_tensor("xT", [DA, S], F32, kind="ExternalInput")
    xcT = nc.dram_tensor("xcT", [DA, NCAP], F32, kind="ExternalInput")
    xTh = nc.dram_tensor("xTh", [DA, S], F16, kind="ExternalInput")
    wq4 = nc.dram_tensor("wq4", [DA, HPC * 128], F16, kind="ExternalInput")
    wqf = nc.dram_tensor("wqf", [DA, D], F32, kind="ExternalInput")
    wkf = nc.dram_tensor("wkf", [DA, D], F32, kind="ExternalInput")
    wv4 = nc.dram_tensor("wv4", [DA, HPC * 128], F16, kind="ExternalInput")
    woh = nc.dram_tensor("woh", [DH, D], F16, kind="ExternalInput")
    ci_col = nc.dram_tensor("ci_col", [NCAP, 1], F32, kind="ExternalInput")
    pada = nc.dram_tensor("pada", [128, NT], F32, kind="ExternalInput")
    w01d = nc.dram_tensor("w01", [3, 128, 128], F16, kind="ExternalInput")
    e4d = nc.dram_tensor("e4", [HPC, 128], F32, kind="ExternalInput")
    identd = nc.dram_tensor("ident", [128, 128], F32, kind="ExternalInput")
    outT = nc.dram_tensor("outT", [D, S], F32, kind="ExternalOutput")
    outcT = nc.dram_tensor("outcT", [2, D, NCAP], F32, kind="ExternalOutput")

    with TileContext(nc) as tc:
        with (
            tc.tile_pool(name="const", bufs=1) as cst,
            tc.tile_pool(name="persist", bufs=1) as per,
            tc.tile_pool(name="mbig", bufs=1) as mbig,
            tc.tile_pool(name="alp", bufs=1) as alp,
            tc.tile_pool(name="wrk", bufs=3) as wrk,
        ):
            # ---------- small constants ----------
            wq4_sb, wqf_sb, wkf_sb, wv4_sb = [], [], [], []
            for k, (lo, hi) in enumerate(((0, 128), (128, 256), (256, 257))):
                p = hi - lo
                t4 = cst.tile([p, HPC * 128], F16, tag=f"wq4_{k}",
                              name=f"wq4_{k}")
                tq = cst.tile([p, D], F32, tag=f"wqf_{k}", name=f"wqf_{k}")
                tk = cst.tile([p, D], F32, tag=f"wkf_{k}", name=f"wkf_{k}")
                tv = cst.tile([p, HPC * 128], F16, tag=f"wv4_{k}",
                              name=f"wv4_{k}")
                nc.sync.dma_start(t4[:], wq4[lo:hi, :])
                nc.sync.dma_start(tq[:], wqf[lo:hi, :])
                nc.sync.dma_start(tk[:], wkf[lo:hi, :])
                nc.sync.dma_start(tv[:], wv4[lo:hi, :])
                wq4_sb.append(t4)
                wqf_sb.append(tq)
                wkf_sb.append(tk)
                wv4_sb.append(tv)
            woh_sb = cst.tile([DH, D], F16, tag="woh")
            nc.sync.dma_start(woh_sb[:], woh[:])
            woh_p = []
            for hp in range(2):
                t = cst.tile([64, D], F16, tag=f"wohp{hp}", name=f"wohp{hp}")
                nc.sync.dma_start(t[:], woh[hp * 64:(hp + 1) * 64, :])
                woh_p.append(t)

            ci_sb = []
            for m in range(NM):
                t = cst.tile([128, 1], F32, tag=f"ci{m}", name=f"ci{m}")
                nc.sync.dma_start(t[:], ci_col[m * 128:(m + 1) * 128, :])
                ci_sb.append(t)
            pada_sb = cst.tile([128, NT], F32, tag="pada")
            nc.sync.dma_start(pada_sb[:], pada[:])

            w01_sb = []
            for k in range(3):
                t = cst.tile([128, 1, 128], F16, tag=f"w01{k}", name=f"w01{k}")
                nc.sync.dma_start(t[:, 0, :], w01d[k])
                w01_sb.append(t)
            e4_sb = cst.tile([HPC, 128], F32, tag="e4")
            nc.sync.dma_start(e4_sb[:], e4d[:])
            e2_sb = []
            for hp in range(2):
                t = cst.tile([2, 64], F32, tag=f"e2_{hp}", name=f"e2_{hp}")
                nc.sync.dma_start(
                    t[:], e4d[2 * hp:2 * hp + 2, 64 * hp:64 * hp + 64])
                e2_sb.append(t)
            ident_sb = cst.tile([128, 128], F32, tag="ident")
            nc.sync.dma_start(ident_sb[:], identd[:])

            ones128 = cst.tile([128, 1], F32, tag="ones128")
            nc.vector.memset(ones128[:], 1.0)
            ones128h = cst.tile([128, 1], F16, tag="ones128h")
            nc.vector.memset(ones128h[:], 1.0)
            ones1 = cst.tile([1, 128], F32, tag="ones1")
            nc.vector.memset(ones1[:], 1.0)

            # persistent activations
            q4a = per.tile([128, HPC, S], F16, tag="q4a")
            kft = [per.tile([128, S], F32, tag=f"kft{m}", name=f"kft{m}")
                   for m in range(2)]
            kfth = per.tile([128, S], F16, tag="kfth")
            qct = [per.tile([128, NCAP], F32, tag=f"qct{m}", name=f"qct{m}")
                   for m in range(2)]
            qc4 = per.tile([128, HPC, NCAP], F16, tag="qc4")
            v4 = [per.tile([128, HPC * 128], F16, tag=f"v4_{j}",
                           name=f"v4_{j}") for j in range(NT)]
            alT_sb = [alp.tile([128, 1, NCAP], F16, tag=f"alT{j}",
                               name=f"alT{j}") for j in range(NT)]

            with (
                tc.tile_pool(name="load", bufs=1) as ld,
                tc.tile_pool(name="psmm", bufs=2, space="PSUM") as psmm,
                tc.tile_pool(name="bps", bufs=3, space="PSUM") as bps,
                tc.tile_pool(name="bacc", bufs=2, space="PSUM") as bap,
                tc.tile_pool(name="bwork", bufs=2) as bwrk,
            ):
                # ---------- load x ----------
                xt0 = ld.tile([128, S], F32, tag="xt0")
                xt1 = ld.tile([128, S], F32, tag="xt1")
                xt2 = ld.tile([1, S], F32, tag="xt2")
                nc.sync.dma_start(xt0[:], xT[0:128, :])
                nc.sync.dma_start(xt1[:], xT[128:256, :])
                nc.sync.dma_start(xt2[:], xT[256:257, :])
                xct0 = ld.tile([128, NCAP], F32, tag="xct0")
                xct1 = ld.tile([128, NCAP], F32, tag="xct1")
                xct2 = ld.tile([1, NCAP], F32, tag="xct2")
                nc.sync.dma_start(xct0[:], xcT[0:128, :])
                nc.sync.dma_start(xct1[:], xcT[128:256, :])
                nc.sync.dma_start(xct2[:], xcT[256:257, :])
                xts = (xt0, xt1, xt2)
                xcts = (xct0, xct1, xct2)
                xh0 = ld.tile([128, S], F16, tag="xh0")
                xh1 = ld.tile([128, S], F16, tag="xh1")
                xh2 = ld.tile([1, S], F16, tag="xh2")
                nc.sync.dma_start(xh0[:], xTh[0:128, :])
                nc.sync.dma_start(xh1[:], xTh[128:256, :])
                nc.sync.dma_start(xh2[:], xTh[256:257, :])
                xhs = (xh0, xh1, xh2)
                xch = ld.tile([128, HPC, NCAP], F16, tag="xch")
                al_t = ld.tile([128, S], F32, tag="al", name="al_t")

                # ---------- projections ----------
                # q4[h]: per-head zero-padded Q (own heads)
                for h in range(HPC):
                    hsl = bass.ts(h, 128)
                    for c in range(4):
                        ps = psmm.tile([128, 512], F32, tag="mm")
                        sl = bass.ts(c, 512)
                        for k in range(3):
                            nc.tensor.matmul(ps[:], wq4_sb[k][:, hsl],
                                             xhs[k][:, sl],
                                             start=(k == 0), stop=(k == 2))
                        nc.scalar.activation(q4a[:, h, sl], ps[:], AF.Copy)

                # KfT full [2][128, S]
                for m in range(2):
                    msl = bass.ts(m, 128)
                    for c in range(4):
                        ps = psmm.tile([128, 512], F32, tag="mm")
                        sl = bass.ts(c, 512)
                        for k in range(3):
                            nc.tensor.matmul(ps[:], wkf_sb[k][:, msl],
                                             xts[k][:, sl],
                                             start=(k == 0), stop=(k == 2))
                        nc.scalar.activation(kft[m][:, sl], ps[:], AF.Copy)
                        if m == 0:
                            nc.scalar.activation(kfth[:, sl], ps[:], AF.Copy)

                # v4: per-head zero-padded V, natural layout
                for jt in range(NT):
                    sl = bass.ts(jt, 128)
                    ps = psmm.tile([128, 512], F32, tag="mm")
                    for k in range(3):
                        nc.tensor.matmul(ps[:], xhs[k][:, sl], wv4_sb[k][:],
                                         start=(k == 0), stop=(k == 2))
                    nc.scalar.activation(v4[jt][:], ps[:], AF.Copy)

                # QcT full [2][128, NCAP] (routing) + qc4 (per-head padded)
                for m in range(2):
                    msl = bass.ts(m, 128)
                    ps = psmm.tile([128, NCAP], F32, tag="mm")
                    for k in range(3):
                        nc.tensor.matmul(ps[:], wqf_sb[k][:, msl], xcts[k][:],
                                         start=(k == 0), stop=(k == 2))
                    nc.scalar.activation(qct[m][:], ps[:], AF.Copy)
                nc.scalar.activation(xch[:, 0, :], xct0[:], AF.Copy)
                nc.scalar.activation(xch[:, 1, :], xct1[:], AF.Copy)
                for h in range(HPC):
                    hsl = bass.ts(h, 128)
                    ps = psmm.tile([128, NCAP], F32, tag="mm")
                    nc.tensor.matmul(ps[:], wq4_sb[0][:, hsl], xch[:, 0, :],
                                     start=True, stop=False)
                    nc.tensor.matmul(ps[:], wq4_sb[1][:, hsl], xch[:, 1, :],
                                     start=False, stop=True)
                    nc.scalar.activation(qc4[:, h, :], ps[:], AF.Copy)

                if stage >= 2:
                    # ------- routing scores + topk + union mask -------
                    for mt in range(NM):
                        sc = mbig.tile([128, S], F32, tag="sc")
                        msl = bass.ts(mt, 128)
                        for c in range(4):
                            ps = psmm.tile([128, 512], F32, tag="mm")
                            sl = bass.ts(c, 512)
                            nc.tensor.matmul(ps[:], qct[0][:, msl],
                                             kft[0][:, sl],
                                             start=True, stop=False)
                            nc.tensor.matmul(ps[:], qct[1][:, msl],
                                             kft[1][:, sl],
                                             start=False, stop=True)
                            nc.scalar.activation(sc[:, sl], ps[:], AF.Copy)

                        m8a = wrk.tile([128, 8], F32, tag="m8a")
                        m8b = wrk.tile([128, 8], F32, tag="m8b")
                        tmp1 = mbig.tile([128, S], F32, tag="tmp1")
                        nc.vector.max(out=m8a[:], in_=sc[:])
                        nc.vector.match_replace(out=tmp1[:],
                                                in_to_replace=m8a[:],
                                                in_values=sc[:],
                                                imm_value=-1e30)
                        nc.vector.max(out=m8b[:], in_=tmp1[:])
                        # window part: al = ((j - i)^2 <= W^2)
                        nc.gpsimd.iota(tmp1[:], pattern=[[1, S]], base=0,
                                       channel_multiplier=0,
                                       allow_small_or_imprecise_dtypes=True)
                        nc.vector.tensor_scalar(tmp1[:], tmp1[:],
                                                ci_sb[mt][:], None,
                                                op0=ALU.subtract)
                        nc.scalar.activation(tmp1[:], tmp1[:], AF.Square)
                        nc.vector.tensor_scalar(al_t[:], tmp1[:],
                                                float(WINDOW * WINDOW), None,
                                                op0=ALU.is_le)
                        # union with guided: al = max(al, sc >= t16)
                        nc.vector.scalar_tensor_tensor(
                            out=al_t[:], in0=sc[:],
                            scalar=m8b[:, 7:8],
                            in1=al_t[:], op0=ALU.is_ge, op1=ALU.max)

                        # transpose allowed-mask into [j, i] tiles
                        for jt in range(NT):
                            jsl = bass.ts(jt, 128)
                            psal = psmm.tile([128, 128], F32, tag="mm")
                            nc.tensor.transpose(psal[:], al_t[:, jsl],
                                                ident_sb[:])
                            nc.scalar.activation(
                                alT_sb[jt][:, 0, bass.ts(mt, 128)],
                                psal[:], AF.Copy)

                if stage >= 3:
                    # ------- banded window attention (all rows) -------
                    for it in range(NT):
                        r0 = it * 128
                        if it == 0:
                            subs = [(0, 1), (1, 2)]
                        elif it == NT - 1:
                            subs = [(it - 1, 0), (it, 1)]
                        else:
                            subs = [(it - 1, 0), (it, 1), (it + 1, 2)]

                        bctx = bap.tile([128, 128], F32, tag="bctx")
                        bsums = bap.tile([1, HPC, 128], F32, tag="bsums", bufs=1)
                        nsub = len(subs)
                        for si, (jt, wk_id) in enumerate(subs):
                            jsl = bass.ts(jt, 128)
                            ps = bps.tile([128, HPC, 128], F32, tag="bsc")
                            nc.tensor.matmul(
                                ps[:], kfth[:, jsl],
                                q4a[:, :, bass.ts(it, 128)],
                                start=True, stop=True)
                            e = bwrk.tile([128, HPC, 128], F16, tag="be")
                            nc.scalar.activation(e[:], ps[:], AF.Exp)
                            em = bwrk.tile([128, HPC, 128], F16, tag="bem")
                            nc.vector.scalar_tensor_tensor(
                                out=em[:], in0=e[:],
                                scalar=pada_sb[:, jt:jt + 1],
                                in1=w01_sb[wk_id][:].to_broadcast(
                                    (128, HPC, 128)),
                                op0=ALU.mult, op1=ALU.mult)
                            st = (si == 0)
                            sp = (si == nsub - 1)
                            nc.tensor.matmul(
                                bsums[:].rearrange("a h n -> a (h n)"),
                                ones128h[:],
                                em[:].rearrange("p h n -> p (h n)"),
                                start=st, stop=sp, skip_group_check=True)
                            for h in range(HPC):
                                nc.tensor.matmul(
                                    bctx[:], v4[jt][:, bass.ts(h, 128)],
                                    em[:, h, :],
                                    start=(st and h == 0),
                                    stop=(sp and h == HPC - 1),
                                    skip_group_check=True)

                        r1 = bwrk.tile([1, HPC, 128], F32, tag="br1")
                        nc.vector.reciprocal(r1[:], bsums[:])
                        r4 = bwrk.tile([HPC, 128], F32, tag="br4")
                        nc.sync.dma_start(r4[:], r1[0:1, :, :])
                        psrb = psmm.tile([128, 128], F32, tag="mm")
                        nc.tensor.matmul(psrb[:], e4_sb[:], r4[:],
                                         start=True, stop=True)
                        rb_sb = bwrk.tile([128, 128], F32, tag="brb")
                        nc.scalar.activation(rb_sb[:], psrb[:], AF.Copy)
                        ctx_sb = bwrk.tile([128, 128], F16, tag="bctxs")
                        nc.vector.tensor_mul(ctx_sb[:], bctx[:], rb_sb[:])
                        pso = psmm.tile([128, 2, 128], F32, tag="mm")
                        for m in range(2):
                            nc.tensor.matmul(pso[:, m, :],
                                             woh_sb[:, bass.ts(m, 128)],
                                             ctx_sb[:], start=True, stop=True)
                        osb = bwrk.tile([128, 2, 128], F32, tag="osb")
                        nc.scalar.activation(osb[:], pso[:], AF.Copy)
                        for m in range(2):
                            nc.sync.dma_start(
                                outT[m * 128:(m + 1) * 128, r0:r0 + 128],
                                osb[:, m, :])

            if stage >= 4:
                # ---------- caller dense attention (two head-pair passes,
                # smaller PSUM footprint -> double-buffered scores) ----------
                with (
                    tc.tile_pool(name="cps", bufs=2, space="PSUM") as cps,
                    tc.tile_pool(name="cacc", bufs=1, space="PSUM") as cacc,
                    tc.tile_pool(name="cwork", bufs=3) as cwrk,
                ):
                    for hp in range(2):
                        cctx = cacc.tile([64, NCAP], F32, tag="cctx",
                                         name=f"cctx{hp}")
                        csums = cacc.tile([1, 2, 512], F32, tag="csums",
                                          name=f"csums{hp}")
                        for jt in range(NT):
                            jsl = bass.ts(jt, 128)
                            st = (jt == 0)
                            sp = (jt == NT - 1)
                            ps = cps.tile([128, 2, 512], F32, tag="csc")
                            for i in range(2):
                                h = hp * 2 + i
                                nc.tensor.matmul(
                                    ps[:, i, 0:NCAP], kfth[:, jsl],
                                    qc4[:, h, :], start=True, stop=True)
                            e = cwrk.tile([128, 2, NCAP], F16, tag="ce")
                            for i in range(2):
                                nc.scalar.activation(e[:, i, :],
                                                     ps[:, i, 0:NCAP], AF.Exp)
                            em = cwrk.tile([128, 2, NCAP], F16, tag="cem")
                            nc.vector.scalar_tensor_tensor(
                                out=em[:], in0=e[:],
                                scalar=pada_sb[:, jt:jt + 1],
                                in1=alT_sb[jt][:].to_broadcast((128, 2, NCAP)),
                                op0=ALU.mult, op1=ALU.mult)
                            emf = em[:].rearrange("p h n -> p (h n)")
                            nc.tensor.matmul(
                                csums[0:1, 0, :], ones128h[:], emf[:, 0:512],
                                start=st, stop=sp, skip_group_check=True)
                            nc.tensor.matmul(
                                csums[0:1, 1, 0:256], ones128h[:],
                                emf[:, 512:768],
                                start=st, stop=sp, skip_group_check=True)
                            for i in range(2):
                                h = hp * 2 + i
                                lo = h * 128 + hp * 64
                                nc.tensor.matmul(
                                    cctx[:], v4[jt][:, lo:lo + 64],
                                    em[:, i, :],
                                    start=(st and i == 0),
                                    stop=(sp and i == 1),
                                    skip_group_check=True)

                        cr1 = cwrk.tile([1, 2, 512], F32, tag="cr1")
                        nc.vector.reciprocal(
                            cr1[:].rearrange("a c n -> a (c n)")[:, 0:768],
                            csums[:].rearrange("a c n -> a (c n)")[:, 0:768])
                        r4c = cwrk.tile([2, NCAP], F32, tag="cr4")
                        nc.sync.dma_start(
                            r4c[:],
                            cr1[0:1, :, :].rearrange(
                                "a c n -> a (c n)")[:, 0:2 * NCAP])
                        pscrb = cps.tile([128, 2, 512], F32, tag="csc")
                        nc.tensor.matmul(
                            pscrb[0:64, 0, 0:NCAP], e2_sb[hp][:], r4c[:],
                            start=True, stop=True)
                        crb_sb = cwrk.tile([64, NCAP], F32, tag="crb")
                        nc.scalar.activation(crb_sb[:], pscrb[0:64, 0, 0:NCAP],
                                             AF.Copy)
                        cctx_sb = cwrk.tile([64, NCAP], F16, tag="cctxs")
                        nc.vector.tensor_mul(cctx_sb[:], cctx[:], crb_sb[:])
                        psoc = cps.tile([128, 2, 512], F32, tag="csc")
                        for m in range(2):
                            nc.tensor.matmul(psoc[:, m, 0:NCAP],
                                             woh_p[hp][:, bass.ts(m, 128)],
                                             cctx_sb[:], start=True, stop=True)
                        ocsb = cwrk.tile([128, 2, NCAP], F32, tag="ocsb")
                        for m in range(2):
                            nc.scalar.activation(ocsb[:, m, :],
                                                 psoc[:, m, 0:NCAP], AF.Copy)
                            nc.sync.dma_start(
                                outcT[hp, m * 128:(m + 1) * 128, :],
                                ocsb[:, m, :])

    nc.compile()
    nc.finalize()
    return nc


_NC_CACHE = None


def _get_program():
    global _NC_CACHE
    if _NC_CACHE is None:
        _NC_CACHE = _build_program()
    return _NC_CACHE


def _host_prepare(x, Wq, bq, Wk, bk, Wv, bv, Wo, bo, opcode_types, pad_mask):
    """Build per-core input dicts + metadata for unsharding."""
    x = np.ascontiguousarray(np.asarray(x, np.float32))
    Wq = np.asarray(Wq, np.float32)
    bq = np.asarray(bq, np.float32)
    Wk = np.asarray(Wk, np.float32)
    bk = np.asarray(bk, np.float32)
    Wv = np.asarray(Wv, np.float32)
    bv = np.asarray(bv, np.float32)
    Wo = np.asarray(Wo, np.float32)
    opcode = np.asarray(opcode_types)
    pad = np.asarray(pad_mask)

    wq_aug = np.vstack([Wq * SCALE, (bq * SCALE)[None, :]])     # [257, 256]
    wk_aug = np.vstack([Wk, bk[None, :]])
    wv_aug = np.vstack([Wv, bv[None, :]])

    w01 = np.zeros((3, 128, 128), np.float16)
    for k, base in enumerate((-128, 0, 128)):
        pj = np.arange(128)[:, None]
        pi = np.arange(128)[None, :]
        w01[k] = (np.abs(base + pj - pi) <= WINDOW).astype(np.float16)
    e4 = np.zeros((HPC, 128), np.float32)
    for h in range(HPC):
        e4[h, h * DK:(h + 1) * DK] = 1.0
    ident = np.eye(128, dtype=np.float32)

    in_maps = []
    meta = []
    for b in range(B):
        cidx = np.where(opcode[b] == 0)[0]
        nrows = len(cidx)
        if nrows > NCAP:
            raise RuntimeError(f"caller rows {nrows} exceed capacity {NCAP}")
        xc = np.zeros((NCAP, D), np.float32)
        xc[:nrows] = x[b, cidx]
        xc_aug = np.concatenate([xc, np.zeros((NCAP, 1), np.float32)], axis=1)
        xc_aug[:nrows, D] = 1.0
        ci = np.full((NCAP, 1), -1e6, np.float32)
        ci[:nrows, 0] = cidx.astype(np.float32)
        xT_aug = np.concatenate([x[b].T, np.ones((1, S), np.float32)], axis=0)
        pad01 = (pad[b] != 0).astype(np.float32)
        pada_arr = pad01.reshape(NT, 128).T.copy()

        meta.append((cidx, nrows))
        for hg in range(2):
            own = np.arange(hg * DH, (hg + 1) * DH)
            rest = np.setdiff1d(np.arange(D), own)
            perm = np.concatenate([own, rest])
            # per-head zero-padded Q / V weight blocks
            wq4_arr = np.zeros((DA, HPC * 128), np.float32)
            wv4_arr = np.zeros((DA, HPC * 128), np.float32)
            for h in range(HPC):
                csl = slice(hg * DH + h * DK, hg * DH + (h + 1) * DK)
                wq4_arr[:, h * 128 + h * DK:h * 128 + (h + 1) * DK] = \
                    wq_aug[:, csl]
                wv4_arr[:, h * 128 + h * DK:h * 128 + (h + 1) * DK] = \
                    wv_aug[:, csl]
            in_maps.append({
                "xT": np.ascontiguousarray(xT_aug),
                "xTh": np.ascontiguousarray(xT_aug.astype(np.float16)),
                "xcT": np.ascontiguousarray(xc_aug.T),
                "wq4": wq4_arr.astype(np.float16),
                "wqf": np.ascontiguousarray(wq_aug[:, perm]),
                "wkf": np.ascontiguousarray(wk_aug[:, perm]),
                "wv4": wv4_arr.astype(np.float16),
                "woh": np.ascontiguousarray(Wo[own, :].astype(np.float16)),
                "ci_col": ci,
                "pada": np.ascontiguousarray(pada_arr),
                "w01": w01,
                "e4": e4,
                "ident": ident,
            })
    return in_maps, meta


def _assemble(results, meta, bo):
    bo = np.asarray(bo, np.float32)
    out = np.empty((B, S, D), np.float32)
    for b in range(B):
        cidx, nrows = meta[b]
        full = results[2 * b]["outT"].T + results[2 * b + 1]["outT"].T
        if nrows > 0:
            oc = (results[2 * b]["outcT"].sum(axis=0) +
                  results[2 * b + 1]["outcT"].sum(axis=0)).T[:nrows]
            full[cidx] = oc
        out[b] = full + bo[None, :]
    return out


def kernel(x, Wq, bq, Wk, bk, Wv, bv, Wo, bo, opcode_types, pad_mask,
           _trace=False):
    nc = _get_program()
    in_maps, meta = _host_prepare(x, Wq, bq, Wk, bk, Wv, bv, Wo, bo,
                                  opcode_types, pad_mask)
    res = run_bass_kernel_spmd(nc, in_maps, core_ids=list(range(8)),
                               trace=_trace)
    out = _assemble(res.results, meta, bo)
    if _trace:
        kernel.last_exec_time_ns = res.exec_time_ns
        kernel.last_results = res
    return out
